# revision 1
# baseline (speedup 1.0000x reference)
"""Trainium2 Bass kernel v4: instruction-lean baseline restructure.

Same math as the baseline; wall time on this rig scales with engine
instruction count (~40-250us/inst), so v4 cuts per-2048-edge-batch
instructions from ~55 to ~31:
- host precomputes s=s1+s2, p=p1+p2 and folds w1/w2 into the dst table
  (tables [s|w|x|0] / [p|w*w2|x*w1|0]): c-add is 1 DVE op, heads are
  mult+reduce (2 ops)  [6 -> 3]
- all gather indices loaded in ONE upfront DMA  [-2 DMA/batch]
- the 8 pair-transposes write one PSUM tile -> ONE lrelu, TWO wide W_mlp
  matmuls, TWO l2 activations  [24 -> 13]
- final + const via ACT Identity bias (no DVE scalar-add)
Gathers all on SWDGE queue 0 (multi-queue round-robin measured 5x slower).
"""

from contextlib import ExitStack

import numpy as np

import concourse.bacc as bacc
import concourse.mybir as mybir
import concourse.tile as tile
from concourse.masks import make_identity

FP16 = mybir.dt.float16
F32 = mybir.dt.float32
I16 = mybir.dt.int16

NCORES = 8
B = 2048          # edges per compute batch
BG = 1024         # edges per dma_gather (descriptor ring limit)
J = B // 128      # 16 slots
D = 64

_prog_cache = {}

# gather position i -> out stream position q
_I = np.arange(B)
_QPERM = (_I % 128) * (B // 128) + (B // 1024) * 4 * (_I // 1024) + (_I // 128) % 8


def _wrap_idx_chunk(idx):
    """[1024] int -> [128, 64] int16 (wrap 16 partitions, replicate 8x)."""
    w = idx.reshape(-1, 16).T.astype(np.int16)  # [16, 64]
    return np.tile(w, (8, 1))


def _build_program(nb, nhalf, reps=1, variant="full",
                   act=mybir.ActivationFunctionType.Lrelu):
    totb = sum(nb)
    nc = bacc.Bacc(
        "TRN2",
        target_bir_lowering=False,
        debug=False,
        enable_asserts=False,
        num_swdge_queues=1,
    )
    src_lo = nc.dram_tensor("src_lo", [nhalf, 256], FP16, kind="ExternalInput").ap()
    src_hi = nc.dram_tensor("src_hi", [nhalf, 256], FP16, kind="ExternalInput").ap()
    dst_lo = nc.dram_tensor("dst_lo", [nhalf, 256], FP16, kind="ExternalInput").ap()
    dst_hi = nc.dram_tensor("dst_hi", [nhalf, 256], FP16, kind="ExternalInput").ap()
    # per batch: 128 cols src idx (2 gathers) + 128 cols dst idx
    idx_d = nc.dram_tensor("idx", [128, totb * 256], I16, kind="ExternalInput").ap()
    wbd_d = nc.dram_tensor("wbd", [128, 128], FP16, kind="ExternalInput").ap()
    bm2_d = nc.dram_tensor("bm2", [128, 1], F32, kind="ExternalInput").ap()
    wlp_d = nc.dram_tensor("wlp", [128, 2], FP16, kind="ExternalInput").ap()
    kb_d = nc.dram_tensor("kb", [128, 1], F32, kind="ExternalInput").ap()
    out_d = nc.dram_tensor("out", [totb * 128, 16], F32, kind="ExternalOutput").ap()

    s_tabs = [src_lo, src_lo, src_hi, src_hi]
    d_tabs = [dst_lo, dst_hi, dst_lo, dst_hi]

    with tile.TileContext(nc) as tc, ExitStack() as ctx:
        const = ctx.enter_context(tc.tile_pool(name="const", bufs=1))
        idx_t = const.tile([128, totb * 256], I16)
        nc.sync.dma_start(idx_t[:], idx_d[:])
        ident = const.tile([128, 128], FP16)
        make_identity(nc, ident[:])
        wbd_t = const.tile([128, 128], FP16)
        nc.sync.dma_start(wbd_t[:], wbd_d[:])
        bm2_t = const.tile([128, 1], F32)
        nc.sync.dma_start(bm2_t[:], bm2_d[:])
        wlp_t = const.tile([128, 2], FP16)
        nc.sync.dma_start(wlp_t[:], wlp_d[:])
        kb_t = const.tile([128, 1], F32)
        nc.sync.dma_start(kb_t[:], kb_d[:])

        gp = ctx.enter_context(tc.tile_pool(name="gath", bufs=3))
        cp = ctx.enter_context(tc.tile_pool(name="csum", bufs=2))
        up = ctx.enter_context(tc.tile_pool(name="umul", bufs=2))
        urp = ctx.enter_context(tc.tile_pool(name="ured", bufs=2))
        atp = ctx.enter_context(tc.tile_pool(name="at", bufs=2))
        l2p = ctx.enter_context(tc.tile_pool(name="l2", bufs=2))
        outp = ctx.enter_context(tc.tile_pool(name="outs", bufs=3))
        ps_t = ctx.enter_context(tc.tile_pool(name="ps_t", bufs=2, space="PSUM"))
        ps_h = ctx.enter_context(tc.tile_pool(name="ps_h", bufs=2, space="PSUM"))
        ps_e = ctx.enter_context(tc.tile_pool(name="ps_e", bufs=2, space="PSUM"))

        for rep in range(reps):
            t = 0
            for k in range(4):
                for _ in range(nb[k]):
                    S = gp.tile([128, J, 256], FP16, tag="S")
                    Dt = gp.tile([128, J, 256], FP16, tag="D")
                    ib = t * 256
                    for h in range(2 if variant != "compute" else 0):
                        nc.gpsimd.dma_gather(
                            out_ap=S[:, h * 8:(h + 1) * 8, :], in_ap=s_tabs[k][:],
                            idxs_ap=idx_t[:, ib + h * 64:ib + (h + 1) * 64],
                            num_idxs=BG, num_idxs_reg=BG, elem_size=256,
                            queue_num=0,
                        )
                        nc.gpsimd.dma_gather(
                            out_ap=Dt[:, h * 8:(h + 1) * 8, :], in_ap=d_tabs[k][:],
                            idxs_ap=idx_t[:, ib + 128 + h * 64:ib + 128 + (h + 1) * 64],
                            num_idxs=BG, num_idxs_reg=BG, elem_size=256,
                            queue_num=0,
                        )
                    if variant == "gather":
                        t += 1
                        continue

                    # c = s[src] + p[dst]   [128, J, 64]
                    c = cp.tile([128, J, D], FP16, tag="c")
                    nc.vector.tensor_tensor(c[:], S[:, :, 0:64], Dt[:, :, 0:64],
                                            op=mybir.AluOpType.add)
                    # heads: u = sum([w|x]_src * [w~|x~]_dst)  [128, J]
                    um = up.tile([128, J, 128], FP16, tag="um")
                    nc.vector.tensor_tensor(um[:], S[:, :, 64:192],
                                            Dt[:, :, 64:192],
                                            op=mybir.AluOpType.mult)
                    ur = urp.tile([128, J], F32, tag="ur")
                    nc.vector.tensor_reduce(ur[:], um[:],
                                            axis=mybir.AxisListType.X,
                                            op=mybir.AluOpType.add)

                    # 8 pair-transposes into ONE psum tile, one lrelu,
                    # two wide W matmuls, two l2 activations
                    pca = ps_t.tile([128, 8, 128], FP16, tag="pca")
                    for u in range(8):
                        nc.tensor.matmul(pca[:, u, :],
                                         lhsT=c[:, 2 * u:2 * u + 2, :],
                                         rhs=ident[:], is_transpose=True,
                                         start=True, stop=True)
                    ata = atp.tile([128, 8, 128], FP16, tag="ata")
                    nc.scalar.activation(ata[:], pca[:], act, alpha=0.01)
                    l2a = l2p.tile([128, 8, 128], FP16, tag="l2a")
                    for hh in range(2):
                        ph = ps_h.tile([128, 4, 128], F32, tag="ph")
                        nc.tensor.matmul(ph[:], lhsT=wbd_t[:],
                                         rhs=ata[:, 4 * hh:4 * hh + 4, :],
                                         start=True, stop=True)
                        nc.scalar.activation(l2a[:, 4 * hh:4 * hh + 4, :], ph[:],
                                             act, bias=bm2_t[:, 0:1], alpha=0.01)
                    e1 = ps_e.tile([128, J], F32, tag="e1")
                    for u in range(8):
                        nc.tensor.matmul(e1[:, 2 * u:2 * u + 2],
                                         lhsT=l2a[:, u, :], rhs=wlp_t[:],
                                         start=True, stop=True)

                    os1 = outp.tile([128, J], F32, tag="os1")
                    nc.vector.tensor_tensor(os1[:], e1[:], ur[:],
                                            op=mybir.AluOpType.add)
                    ot = outp.tile([128, J], F32, tag="ot")
                    nc.scalar.activation(ot[:], os1[:],
                                         mybir.ActivationFunctionType.Identity,
                                         bias=kb_t[:, 0:1])
                    nc.sync.dma_start(out_d[t * 128:(t + 1) * 128, :], ot[:])
                    t += 1

    nc.compile()
    return nc


def _prep(inputs):
    src = np.asarray(inputs["src"]).astype(np.int64).ravel()
    dst = np.asarray(inputs["dst"]).astype(np.int64).ravel()
    s = (np.asarray(inputs["s1"], np.float32)
         + np.asarray(inputs["s2"], np.float32))
    p = (np.asarray(inputs["p1"], np.float32)
         + np.asarray(inputs["p2"], np.float32))
    x = np.asarray(inputs["x"], np.float32)
    w = np.asarray(inputs["w"], np.float32)
    w1 = np.asarray(inputs["w1"], np.float32).ravel()
    w2 = np.asarray(inputs["w2"], np.float32).ravel()

    E = src.shape[0]
    N = s.shape[0]
    assert E % NCORES == 0
    epc = E // NCORES
    nhalf = (N + 1) // 2

    z = np.zeros_like(x)
    src_tab = np.concatenate([s, w, x, z], axis=1).astype(np.float16)
    dst_tab = np.concatenate([p, w * w2[None, :], x * w1[None, :], z],
                             axis=1).astype(np.float16)
    if N < 2 * nhalf:
        pad = np.zeros((2 * nhalf - N, 256), np.float16)
        src_tab = np.vstack([src_tab, pad])
        dst_tab = np.vstack([dst_tab, pad])

    per_core = []
    counts = np.zeros((NCORES, 4), np.int64)
    for c in range(NCORES):
        sc = src[c * epc:(c + 1) * epc]
        dc = dst[c * epc:(c + 1) * epc]
        b = (sc >= nhalf) * 2 + (dc >= nhalf)
        ords = [np.flatnonzero(b == k) for k in range(4)]
        counts[c] = [len(o) for o in ords]
        per_core.append((sc, dc, ords))

    nb = [int(-(-counts[:, k].max() // B)) for k in range(4)]
    totb = sum(nb)

    idx_all = np.zeros((NCORES, 128, totb * 256), np.int16)
    order_all = np.full((NCORES, totb * B), -1, np.int64)

    for c in range(NCORES):
        sc, dc, ords = per_core[c]
        t = 0
        pos = 0
        for k in range(4):
            ids = ords[k]
            cap = nb[k] * B
            se = np.zeros(cap, np.int64)
            de = np.zeros(cap, np.int64)
            se[:len(ids)] = sc[ids] - (nhalf if k >= 2 else 0)
            de[:len(ids)] = dc[ids] - (nhalf if k % 2 == 1 else 0)
            order_all[c, pos:pos + len(ids)] = ids
            pos += cap
            for bi in range(nb[k]):
                seg_s = se[bi * B + _QPERM]
                seg_d = de[bi * B + _QPERM]
                ib = t * 256
                for h in range(2):
                    sl = slice(h * BG, (h + 1) * BG)
                    idx_all[c, :, ib + h * 64:ib + (h + 1) * 64] = (
                        _wrap_idx_chunk(seg_s[sl]))
                    idx_all[c, :, ib + 128 + h * 64:ib + 128 + (h + 1) * 64] = (
                        _wrap_idx_chunk(seg_d[sl]))
                t += 1

    W_mlp = np.asarray(inputs["W_mlp"], np.float32)
    b_mlp = np.asarray(inputs["b_mlp"], np.float32).ravel()
    wL = np.asarray(inputs["wL"], np.float32).ravel()
    kb = (float(np.asarray(inputs["bL"]).ravel()[0])
          + float(np.asarray(inputs["b1"]).ravel()[0])
          + float(np.asarray(inputs["b2"]).ravel()[0]))

    wbd = np.zeros((128, 128), np.float16)
    wbd[:64, :64] = W_mlp.astype(np.float16)
    wbd[64:, 64:] = W_mlp.astype(np.float16)
    wlp = np.zeros((128, 2), np.float16)
    wlp[:64, 0] = wL.astype(np.float16)
    wlp[64:, 1] = wL.astype(np.float16)
    bm2 = np.concatenate([b_mlp, b_mlp]).astype(np.float32).reshape(128, 1)

    weights = dict(
        wbd=wbd, bm2=bm2, wlp=wlp,
        kb=np.full((128, 1), kb, np.float32),
    )
    tabs = dict(
        src_lo=np.ascontiguousarray(src_tab[:nhalf]),
        src_hi=np.ascontiguousarray(src_tab[nhalf:]),
        dst_lo=np.ascontiguousarray(dst_tab[:nhalf]),
        dst_hi=np.ascontiguousarray(dst_tab[nhalf:]),
    )
    return tuple(nb), nhalf, epc, E, tabs, weights, idx_all, order_all


def run(inputs, **spmd_kwargs):
    from concourse.bass_utils import run_bass_kernel_spmd

    nb, nhalf, epc, E, tabs, weights, idx_all, order_all = _prep(inputs)

    key = (nb, nhalf)
    if key not in _prog_cache:
        _prog_cache[key] = _build_program(list(nb), nhalf)
    nc = _prog_cache[key]

    in_maps = []
    for c in range(NCORES):
        m = dict(tabs)
        m.update(weights)
        m["idx"] = idx_all[c]
        in_maps.append(m)

    res = run_bass_kernel_spmd(nc, in_maps, list(range(NCORES)), **spmd_kwargs)

    out = np.empty((E, 1), np.float32)
    for c in range(NCORES):
        oc = np.asarray(res.results[c]["out"], np.float32).reshape(-1)
        order = order_all[c]
        valid = order >= 0
        out[c * epc + order[valid], 0] = oc[valid]
    return out, res


def kernel(**inputs) -> np.ndarray:
    out, _ = run(inputs)
    return out



# revision 5
# speedup vs baseline: 1.1610x; 1.1610x over previous
"""Trainium2 Bass kernel v8: DMA-transpose pipeline.

Rig findings: every engine instruction costs ~60-150us fixed (size barely
matters); gathers (~360us each, 1024-idx ring max) largely overlap compute;
so minimize total instruction count. dma_start_transpose does a blocked
transpose out[q,b,j] = in[j,128b+q] of an entire [128,N] tile in ONE DMA
instruction (verified on device) — replacing v4's 8-per-2048-edge PE
transposes AND enabling an edge-major final reduce that replaces the
8-per-2048-edge wL matmuls.

Per 4096-edge batch (22 instructions vs v4's 62):
  8 gathers -> S,D [128,32,256] (edge-major, row [s|w|x|0])
  c = S.c + D.c                      1 DVE   [128,32,64]
  a = lrelu(c)                       1 ACT   (lrelu commutes with transpose)
  ata = blockT(a)                    1 DMA-transpose -> 2-slot feature-major
  z = wbd^T @ ata                    4 PE    (512-col f32 PSUM banks)
  l2a = lrelu(z + b)                 1 ACT   (4-bank PSUM read)
  l2e = blockT(l2a)                  1 DMA-transpose -> edge-major
  l2w = l2e * [wL|wL]                1 DVE
  rL = reduce64(l2w)                 1 DVE   -> [128, 32] per-edge MLP term
  um = S.wx * D.wx                   1 DVE   [128,32,128]
  ur = reduce128(um)                 1 DVE   -> [128, 32] heads term
  os = rL + ur                       1 DVE
  ot = os + (bL+b1+b2)               1 ACT
  out DMA                            1
"""

from contextlib import ExitStack

import numpy as np

import concourse.bacc as bacc
import concourse.mybir as mybir
import concourse.tile as tile

FP16 = mybir.dt.float16
F32 = mybir.dt.float32
I16 = mybir.dt.int16

NCORES = 8
B = 4096          # edges per compute batch
BG = 1024         # edges per dma_gather (descriptor ring limit)
NG = B // BG      # gathers per side per batch (4)
J = B // 128      # slots (32)
NBLK = B // 128 // 2  # 128-col blocks in 2-slot layout (16)
D = 64

_prog_cache = {}


def _wrap_idx_chunk(idx):
    """[1024] int -> [128, 64] int16 (wrap 16 partitions, replicate 8x)."""
    w = idx.reshape(-1, 16).T.astype(np.int16)  # [16, 64]
    return np.tile(w, (8, 1))


def _build_program(nb, nhalf, reps=1, variant="full", queues=1):
    totb = sum(nb)
    nc = bacc.Bacc(
        "TRN2",
        target_bir_lowering=False,
        debug=False,
        enable_asserts=False,
        num_swdge_queues=queues,
    )
    src_lo = nc.dram_tensor("src_lo", [nhalf, 256], FP16, kind="ExternalInput").ap()
    src_hi = nc.dram_tensor("src_hi", [nhalf, 256], FP16, kind="ExternalInput").ap()
    dst_lo = nc.dram_tensor("dst_lo", [nhalf, 256], FP16, kind="ExternalInput").ap()
    dst_hi = nc.dram_tensor("dst_hi", [nhalf, 256], FP16, kind="ExternalInput").ap()
    idx_d = nc.dram_tensor("idx", [128, totb * NG * 128], I16,
                           kind="ExternalInput").ap()
    wbd_d = nc.dram_tensor("wbd", [128, 128], FP16, kind="ExternalInput").ap()
    bm2_d = nc.dram_tensor("bm2", [128, 1], F32, kind="ExternalInput").ap()
    wlpw_d = nc.dram_tensor("wlpw", [128, NBLK * 128], FP16,
                            kind="ExternalInput").ap()
    kb_d = nc.dram_tensor("kb", [128, 1], F32, kind="ExternalInput").ap()
    out_d = nc.dram_tensor("out", [totb * 128, J], F32,
                           kind="ExternalOutput").ap()

    s_tabs = [src_lo, src_lo, src_hi, src_hi]
    d_tabs = [dst_lo, dst_hi, dst_lo, dst_hi]
    if queues == 4:
        qs_map = [0, 1, 0, 1]
        qd_map = [2, 3, 2, 3]
    elif queues == 2:
        qs_map = [0, 0, 0, 0]
        qd_map = [1, 1, 1, 1]
    else:
        qs_map = [0, 0, 0, 0]
        qd_map = [0, 0, 0, 0]

    with tile.TileContext(nc) as tc, ExitStack() as ctx:
        const = ctx.enter_context(tc.tile_pool(name="const", bufs=1))
        idx_t = const.tile([128, totb * NG * 128], I16)
        nc.sync.dma_start(idx_t[:], idx_d[:])
        wbd_t = const.tile([128, 128], FP16)
        nc.sync.dma_start(wbd_t[:], wbd_d[:])
        bm2_t = const.tile([128, 1], F32)
        nc.sync.dma_start(bm2_t[:], bm2_d[:])
        wlpw_t = const.tile([128, NBLK * 128], FP16)
        nc.sync.dma_start(wlpw_t[:], wlpw_d[:])
        kb_t = const.tile([128, 1], F32)
        nc.sync.dma_start(kb_t[:], kb_d[:])

        gp = ctx.enter_context(tc.tile_pool(name="gath", bufs=2))
        cp = ctx.enter_context(tc.tile_pool(name="c", bufs=2))
        apl = ctx.enter_context(tc.tile_pool(name="a", bufs=2))
        atp = ctx.enter_context(tc.tile_pool(name="ata", bufs=2))
        l2p = ctx.enter_context(tc.tile_pool(name="l2", bufs=2))
        lep = ctx.enter_context(tc.tile_pool(name="l2e", bufs=2))
        lwp = ctx.enter_context(tc.tile_pool(name="l2w", bufs=2))
        ump = ctx.enter_context(tc.tile_pool(name="um", bufs=2))
        smp = ctx.enter_context(tc.tile_pool(name="sm", bufs=3))
        outp = ctx.enter_context(tc.tile_pool(name="outs", bufs=3))
        ps_z = ctx.enter_context(tc.tile_pool(name="ps_z", bufs=2, space="PSUM"))

        for rep in range(reps):
            t = 0
            for k in range(4):
                for _ in range(nb[k]):
                    Sg = gp.tile([128, J, 256], FP16, tag="S")
                    Dg = gp.tile([128, J, 256], FP16, tag="D")
                    ib = t * NG * 128
                    for h in range(NG if variant != "compute" or t == 0
                                   else 0):
                        nc.gpsimd.dma_gather(
                            out_ap=Sg[:, h * 8:(h + 1) * 8, :],
                            in_ap=s_tabs[k][:],
                            idxs_ap=idx_t[:, ib + h * 64:ib + (h + 1) * 64],
                            num_idxs=BG, num_idxs_reg=BG, elem_size=256,
                            queue_num=qs_map[h],
                        )
                        nc.gpsimd.dma_gather(
                            out_ap=Dg[:, h * 8:(h + 1) * 8, :],
                            in_ap=d_tabs[k][:],
                            idxs_ap=idx_t[:, ib + NG * 64 + h * 64:
                                          ib + NG * 64 + (h + 1) * 64],
                            num_idxs=BG, num_idxs_reg=BG, elem_size=256,
                            queue_num=qd_map[h],
                        )
                    if variant == "gather":
                        t += 1
                        continue

                    # c = s[src] + p[dst]
                    c = cp.tile([128, J, D], FP16, tag="c")
                    nc.vector.tensor_tensor(c[:], Sg[:, :, 0:64],
                                            Dg[:, :, 0:64],
                                            op=mybir.AluOpType.add)
                    # lrelu#1 (edge-major; commutes with transpose)
                    a = apl.tile([128, J * D], FP16, tag="a")
                    nc.scalar.activation(a[:], c[:],
                                         mybir.ActivationFunctionType.Lrelu,
                                         alpha=0.01)
                    # blocked transpose -> 2-slot feature-major
                    ata = atp.tile([128, NBLK, 128], FP16, tag="ata")
                    nc.sync.dma_start_transpose(ata[:], a[:])
                    # z = wbd^T @ ata, 512 f32 cols per bank
                    zp = ps_z.tile([128, NBLK // 4, 512], F32, tag="zp")
                    for q in range(NBLK // 4):
                        nc.tensor.matmul(zp[:, q, :], lhsT=wbd_t[:],
                                         rhs=ata[:, 4 * q:4 * q + 4, :],
                                         start=True, stop=True)
                    # lrelu#2 + b_mlp (multi-bank PSUM read)
                    l2a = l2p.tile([128, NBLK * 128], FP16, tag="l2a")
                    nc.scalar.activation(l2a[:], zp[:],
                                         mybir.ActivationFunctionType.Lrelu,
                                         bias=bm2_t[:, 0:1], alpha=0.01)
                    # blocked transpose back -> edge-major
                    l2e = lep.tile([128, NBLK, 128], FP16, tag="l2e")
                    nc.sync.dma_start_transpose(l2e[:], l2a[:])
                    # wL weighting + per-edge 64-reduce (slot-even/odd halves)
                    l2w = lwp.tile([128, NBLK, 128], FP16, tag="l2w")
                    nc.vector.tensor_tensor(l2w[:], l2e[:], wlpw_t[:],
                                            op=mybir.AluOpType.mult)
                    rL = smp.tile([128, NBLK, 2], F32, tag="rL")
                    nc.vector.tensor_reduce(rL[:, :, 0], l2w[:, :, 0:64],
                                            axis=mybir.AxisListType.X,
                                            op=mybir.AluOpType.add)
                    nc.vector.tensor_reduce(rL[:, :, 1], l2w[:, :, 64:128],
                                            axis=mybir.AxisListType.X,
                                            op=mybir.AluOpType.add)
                    # heads: u = sum([w|x]_src * [w~|x~]_dst)
                    um = ump.tile([128, J, 128], FP16, tag="um")
                    nc.vector.tensor_tensor(um[:], Sg[:, :, 64:192],
                                            Dg[:, :, 64:192],
                                            op=mybir.AluOpType.mult)
                    ur = smp.tile([128, J], F32, tag="ur")
                    nc.vector.tensor_reduce(ur[:], um[:],
                                            axis=mybir.AxisListType.X,
                                            op=mybir.AluOpType.add)
                    os1 = smp.tile([128, J], F32, tag="os1")
                    nc.vector.tensor_tensor(os1[:], rL[:], ur[:],
                                            op=mybir.AluOpType.add)
                    ot = outp.tile([128, J], F32, tag="ot")
                    nc.scalar.activation(ot[:], os1[:],
                                         mybir.ActivationFunctionType.Identity,
                                         bias=kb_t[:, 0:1])
                    nc.sync.dma_start(out_d[t * 128:(t + 1) * 128, :], ot[:])
                    t += 1

    nc.compile()
    return nc


def _prep(inputs):
    src = np.asarray(inputs["src"]).astype(np.int64).ravel()
    dst = np.asarray(inputs["dst"]).astype(np.int64).ravel()
    s = (np.asarray(inputs["s1"], np.float32)
         + np.asarray(inputs["s2"], np.float32))
    p = (np.asarray(inputs["p1"], np.float32)
         + np.asarray(inputs["p2"], np.float32))
    x = np.asarray(inputs["x"], np.float32)
    w = np.asarray(inputs["w"], np.float32)
    w1 = np.asarray(inputs["w1"], np.float32).ravel()
    w2 = np.asarray(inputs["w2"], np.float32).ravel()

    E = src.shape[0]
    N = s.shape[0]
    assert E % NCORES == 0
    epc = E // NCORES
    nhalf = (N + 1) // 2

    z = np.zeros_like(x)
    src_tab = np.concatenate([s, w, x, z], axis=1).astype(np.float16)
    dst_tab = np.concatenate([p, w * w2[None, :], x * w1[None, :], z],
                             axis=1).astype(np.float16)
    if N < 2 * nhalf:
        pad = np.zeros((2 * nhalf - N, 256), np.float16)
        src_tab = np.vstack([src_tab, pad])
        dst_tab = np.vstack([dst_tab, pad])

    per_core = []
    counts = np.zeros((NCORES, 4), np.int64)
    for c in range(NCORES):
        sc = src[c * epc:(c + 1) * epc]
        dc = dst[c * epc:(c + 1) * epc]
        b = (sc >= nhalf) * 2 + (dc >= nhalf)
        ords = [np.flatnonzero(b == k) for k in range(4)]
        counts[c] = [len(o) for o in ords]
        per_core.append((sc, dc, ords))

    nb = [int(-(-counts[:, k].max() // B)) for k in range(4)]
    totb = sum(nb)

    idx_all = np.zeros((NCORES, 128, totb * NG * 128), np.int16)
    order_all = np.full((NCORES, totb * B), -1, np.int64)

    for c in range(NCORES):
        sc, dc, ords = per_core[c]
        t = 0
        pos = 0
        for k in range(4):
            ids = ords[k]
            cap = nb[k] * B
            se = np.zeros(cap, np.int64)
            de = np.zeros(cap, np.int64)
            se[:len(ids)] = sc[ids] - (nhalf if k >= 2 else 0)
            de[:len(ids)] = dc[ids] - (nhalf if k % 2 == 1 else 0)
            order_all[c, pos:pos + len(ids)] = ids
            pos += cap
            for bi in range(nb[k]):
                ib = t * NG * 128
                for h in range(NG):
                    sl = slice(bi * B + h * BG, bi * B + (h + 1) * BG)
                    idx_all[c, :, ib + h * 64:ib + (h + 1) * 64] = (
                        _wrap_idx_chunk(se[sl]))
                    idx_all[c, :, ib + NG * 64 + h * 64:
                            ib + NG * 64 + (h + 1) * 64] = (
                        _wrap_idx_chunk(de[sl]))
                t += 1

    W_mlp = np.asarray(inputs["W_mlp"], np.float32)
    b_mlp = np.asarray(inputs["b_mlp"], np.float32).ravel()
    wL = np.asarray(inputs["wL"], np.float32).ravel()
    kb = (float(np.asarray(inputs["bL"]).ravel()[0])
          + float(np.asarray(inputs["b1"]).ravel()[0])
          + float(np.asarray(inputs["b2"]).ravel()[0]))

    wbd = np.zeros((128, 128), np.float16)
    wbd[:64, :64] = W_mlp.astype(np.float16)
    wbd[64:, 64:] = W_mlp.astype(np.float16)
    bm2 = np.concatenate([b_mlp, b_mlp]).astype(np.float32).reshape(128, 1)
    wlp2 = np.concatenate([wL, wL]).astype(np.float16)  # [128]
    wlpw = np.tile(wlp2[None, :], (128, NBLK))  # [128, NBLK*128]

    weights = dict(
        wbd=wbd, bm2=bm2, wlpw=wlpw,
        kb=np.full((128, 1), kb, np.float32),
    )
    tabs = dict(
        src_lo=np.ascontiguousarray(src_tab[:nhalf]),
        src_hi=np.ascontiguousarray(src_tab[nhalf:]),
        dst_lo=np.ascontiguousarray(dst_tab[:nhalf]),
        dst_hi=np.ascontiguousarray(dst_tab[nhalf:]),
    )
    return tuple(nb), nhalf, epc, E, tabs, weights, idx_all, order_all


def run(inputs, queues=2, **spmd_kwargs):
    from concourse.bass_utils import run_bass_kernel_spmd

    nb, nhalf, epc, E, tabs, weights, idx_all, order_all = _prep(inputs)

    key = (nb, nhalf, queues)
    if key not in _prog_cache:
        _prog_cache[key] = _build_program(list(nb), nhalf, queues=queues)
    nc = _prog_cache[key]

    in_maps = []
    for c in range(NCORES):
        m = dict(tabs)
        m.update(weights)
        m["idx"] = idx_all[c]
        in_maps.append(m)

    res = run_bass_kernel_spmd(nc, in_maps, list(range(NCORES)), **spmd_kwargs)

    out = np.empty((E, 1), np.float32)
    for c in range(NCORES):
        oc = np.asarray(res.results[c]["out"], np.float32)  # [totb*128, J]
        # edge e of batch t sits at oc[t*128 + e%128, e//128]
        totb = oc.shape[0] // 128
        oc = oc.reshape(totb, 128, J).transpose(0, 2, 1).reshape(-1)
        order = order_all[c]
        valid = order >= 0
        out[c * epc + order[valid], 0] = oc[valid]
    return out, res


def kernel(**inputs) -> np.ndarray:
    out, _ = run(inputs)
    return out


# revision 6
# speedup vs baseline: 4.6172x; 3.9770x over previous
"""Trainium2 Bass kernel v8: DMA-transpose pipeline.

Rig findings: every engine instruction costs ~60-150us fixed (size barely
matters); gathers (~360us each, 1024-idx ring max) largely overlap compute;
so minimize total instruction count. dma_start_transpose does a blocked
transpose out[q,b,j] = in[j,128b+q] of an entire [128,N] tile in ONE DMA
instruction (verified on device) — replacing v4's 8-per-2048-edge PE
transposes AND enabling an edge-major final reduce that replaces the
8-per-2048-edge wL matmuls.

Per 4096-edge batch (22 instructions vs v4's 62):
  8 gathers -> S,D [128,32,256] (edge-major, row [s|w|x|0])
  c = S.c + D.c                      1 DVE   [128,32,64]
  a = lrelu(c)                       1 ACT   (lrelu commutes with transpose)
  ata = blockT(a)                    1 DMA-transpose -> 2-slot feature-major
  z = wbd^T @ ata                    4 PE    (512-col f32 PSUM banks)
  l2a = lrelu(z + b)                 1 ACT   (4-bank PSUM read)
  l2e = blockT(l2a)                  1 DMA-transpose -> edge-major
  l2w = l2e * [wL|wL]                1 DVE
  rL = reduce64(l2w)                 1 DVE   -> [128, 32] per-edge MLP term
  um = S.wx * D.wx                   1 DVE   [128,32,128]
  ur = reduce128(um)                 1 DVE   -> [128, 32] heads term
  os = rL + ur                       1 DVE
  ot = os + (bL+b1+b2)               1 ACT
  out DMA                            1
"""

from contextlib import ExitStack

import numpy as np

import concourse.bacc as bacc
import concourse.mybir as mybir
import concourse.tile as tile

FP16 = mybir.dt.float16
F32 = mybir.dt.float32
I16 = mybir.dt.int16

NCORES = 8
B = 4096          # edges per compute batch
BG = 1024         # edges per dma_gather (descriptor ring limit)
NG = B // BG      # gathers per side per batch (4)
J = B // 128      # slots (32)
NBLK = B // 128 // 2  # 128-col blocks in 2-slot layout (16)
D = 64

_prog_cache = {}


def _wrap_idx_chunk(idx):
    """[1024] int -> [128, 64] int16 (wrap 16 partitions, replicate 8x)."""
    w = idx.reshape(-1, 16).T.astype(np.int16)  # [16, 64]
    return np.tile(w, (8, 1))


def _build_program(nb, nhalf, reps=1, variant="full", queues=1):
    totb = sum(nb)
    nc = bacc.Bacc(
        "TRN2",
        target_bir_lowering=False,
        debug=False,
        enable_asserts=False,
        num_swdge_queues=queues,
    )
    src_lo = nc.dram_tensor("src_lo", [nhalf, 256], FP16, kind="ExternalInput").ap()
    src_hi = nc.dram_tensor("src_hi", [nhalf, 256], FP16, kind="ExternalInput").ap()
    dst_lo = nc.dram_tensor("dst_lo", [nhalf, 256], FP16, kind="ExternalInput").ap()
    dst_hi = nc.dram_tensor("dst_hi", [nhalf, 256], FP16, kind="ExternalInput").ap()
    idx_d = nc.dram_tensor("idx", [128, totb * NG * 128], I16,
                           kind="ExternalInput").ap()
    wbd_d = nc.dram_tensor("wbd", [128, 128], FP16, kind="ExternalInput").ap()
    bm2_d = nc.dram_tensor("bm2", [128, 1], F32, kind="ExternalInput").ap()
    wlpw_d = nc.dram_tensor("wlpw", [128, NBLK * 128], FP16,
                            kind="ExternalInput").ap()
    kb_d = nc.dram_tensor("kb", [128, 1], F32, kind="ExternalInput").ap()
    out_d = nc.dram_tensor("out", [totb * 128, J], F32,
                           kind="ExternalOutput").ap()

    s_tabs = [src_lo, src_lo, src_hi, src_hi]
    d_tabs = [dst_lo, dst_hi, dst_lo, dst_hi]
    if queues == 4:
        qs_map = [0, 1, 0, 1]
        qd_map = [2, 3, 2, 3]
    elif queues == 2:
        qs_map = [0, 0, 0, 0]
        qd_map = [1, 1, 1, 1]
    else:
        qs_map = [0, 0, 0, 0]
        qd_map = [0, 0, 0, 0]

    with tile.TileContext(nc) as tc, ExitStack() as ctx:
        const = ctx.enter_context(tc.tile_pool(name="const", bufs=1))
        idx_t = const.tile([128, totb * NG * 128], I16)
        nc.sync.dma_start(idx_t[:], idx_d[:])
        wbd_t = const.tile([128, 128], FP16)
        nc.sync.dma_start(wbd_t[:], wbd_d[:])
        bm2_t = const.tile([128, 1], F32)
        nc.sync.dma_start(bm2_t[:], bm2_d[:])
        wlpw_t = const.tile([128, NBLK * 128], FP16)
        nc.sync.dma_start(wlpw_t[:], wlpw_d[:])
        kb_t = const.tile([128, 1], F32)
        nc.sync.dma_start(kb_t[:], kb_d[:])

        gp = ctx.enter_context(tc.tile_pool(name="gath", bufs=2))
        cp = ctx.enter_context(tc.tile_pool(name="c", bufs=3))
        apl = ctx.enter_context(tc.tile_pool(name="a", bufs=3))
        atp = ctx.enter_context(tc.tile_pool(name="ata", bufs=3))
        l2p = ctx.enter_context(tc.tile_pool(name="l2", bufs=3))
        lep = ctx.enter_context(tc.tile_pool(name="l2e", bufs=2))
        lwp = ctx.enter_context(tc.tile_pool(name="l2w", bufs=2))
        ump = ctx.enter_context(tc.tile_pool(name="um", bufs=2))
        smp = ctx.enter_context(tc.tile_pool(name="sm", bufs=3))
        outp = ctx.enter_context(tc.tile_pool(name="outs", bufs=3))
        ps_z = ctx.enter_context(tc.tile_pool(name="ps_z", bufs=2, space="PSUM"))

        for rep in range(reps):
            t = 0
            for k in range(4):
                for _ in range(nb[k]):
                    Sg = gp.tile([128, J, 256], FP16, tag="S")
                    Dg = gp.tile([128, J, 256], FP16, tag="D")
                    ib = t * NG * 128
                    for h in range(NG if variant != "compute" or t == 0
                                   else 0):
                        nc.gpsimd.dma_gather(
                            out_ap=Sg[:, h * 8:(h + 1) * 8, :],
                            in_ap=s_tabs[k][:],
                            idxs_ap=idx_t[:, ib + h * 64:ib + (h + 1) * 64],
                            num_idxs=BG, num_idxs_reg=BG, elem_size=256,
                            queue_num=qs_map[h],
                        )
                        nc.gpsimd.dma_gather(
                            out_ap=Dg[:, h * 8:(h + 1) * 8, :],
                            in_ap=d_tabs[k][:],
                            idxs_ap=idx_t[:, ib + NG * 64 + h * 64:
                                          ib + NG * 64 + (h + 1) * 64],
                            num_idxs=BG, num_idxs_reg=BG, elem_size=256,
                            queue_num=qd_map[h],
                        )
                    if variant == "gather":
                        t += 1
                        continue

                    # c = s[src] + p[dst]
                    c = cp.tile([128, J, D], FP16, tag="c")
                    nc.vector.tensor_tensor(c[:], Sg[:, :, 0:64],
                                            Dg[:, :, 0:64],
                                            op=mybir.AluOpType.add)
                    # lrelu#1 (edge-major; commutes with transpose)
                    a = apl.tile([128, J * D], FP16, tag="a")
                    nc.scalar.activation(a[:], c[:],
                                         mybir.ActivationFunctionType.Lrelu,
                                         alpha=0.01)
                    # blocked transpose -> 2-slot feature-major
                    ata = atp.tile([128, NBLK, 128], FP16, tag="ata")
                    nc.sync.dma_start_transpose(ata[:], a[:])
                    # z = wbd^T @ ata, 512 f32 cols per bank
                    zp = ps_z.tile([128, NBLK // 4, 512], F32, tag="zp")
                    for q in range(NBLK // 4):
                        nc.tensor.matmul(zp[:, q, :], lhsT=wbd_t[:],
                                         rhs=ata[:, 4 * q:4 * q + 4, :],
                                         start=True, stop=True)
                    # lrelu#2 + b_mlp (multi-bank PSUM read)
                    l2a = l2p.tile([128, NBLK * 128], FP16, tag="l2a")
                    nc.scalar.activation(l2a[:], zp[:],
                                         mybir.ActivationFunctionType.Lrelu,
                                         bias=bm2_t[:, 0:1], alpha=0.01)
                    # blocked transpose back -> edge-major
                    l2e = lep.tile([128, NBLK, 128], FP16, tag="l2e")
                    nc.sync.dma_start_transpose(l2e[:], l2a[:])
                    # wL weighting + per-edge 64-reduce (slot-even/odd halves)
                    l2w = lwp.tile([128, NBLK, 2, 64], FP16, tag="l2w")
                    nc.vector.tensor_tensor(l2w[:], l2e[:], wlpw_t[:],
                                            op=mybir.AluOpType.mult)
                    rL = smp.tile([128, NBLK, 2], F32, tag="rL")
                    nc.vector.tensor_reduce(rL[:], l2w[:],
                                            axis=mybir.AxisListType.X,
                                            op=mybir.AluOpType.add)
                    # heads: u = sum([w|x]_src * [w~|x~]_dst)
                    um = ump.tile([128, J, 192], FP16, tag="um")
                    nc.vector.tensor_tensor(um[:], Sg[:, :, 64:256],
                                            Dg[:, :, 64:256],
                                            op=mybir.AluOpType.mult)
                    ur = smp.tile([128, J], F32, tag="ur")
                    nc.vector.tensor_reduce(ur[:], um[:],
                                            axis=mybir.AxisListType.X,
                                            op=mybir.AluOpType.add)
                    ot = outp.tile([128, J], F32, tag="ot")
                    nc.vector.tensor_tensor(ot[:], rL[:], ur[:],
                                            op=mybir.AluOpType.add)
                    nc.sync.dma_start(out_d[t * 128:(t + 1) * 128, :], ot[:])
                    t += 1

    nc.compile()
    return nc


def _prep(inputs):
    src = np.asarray(inputs["src"]).astype(np.int64).ravel()
    dst = np.asarray(inputs["dst"]).astype(np.int64).ravel()
    s = (np.asarray(inputs["s1"], np.float32)
         + np.asarray(inputs["s2"], np.float32))
    p = (np.asarray(inputs["p1"], np.float32)
         + np.asarray(inputs["p2"], np.float32))
    x = np.asarray(inputs["x"], np.float32)
    w = np.asarray(inputs["w"], np.float32)
    w1 = np.asarray(inputs["w1"], np.float32).ravel()
    w2 = np.asarray(inputs["w2"], np.float32).ravel()

    E = src.shape[0]
    N = s.shape[0]
    assert E % NCORES == 0
    epc = E // NCORES
    nhalf = (N + 1) // 2

    kb = (float(np.asarray(inputs["bL"]).ravel()[0])
          + float(np.asarray(inputs["b1"]).ravel()[0])
          + float(np.asarray(inputs["b2"]).ravel()[0]))
    z = np.zeros_like(x)
    src_tab = np.concatenate([s, w, x, z], axis=1).astype(np.float16)
    dst_tab = np.concatenate([p, w * w2[None, :], x * w1[None, :], z],
                             axis=1).astype(np.float16)
    src_tab[:, 192] = 1.0
    dst_tab[:, 192] = np.float16(kb)
    if N < 2 * nhalf:
        pad = np.zeros((2 * nhalf - N, 256), np.float16)
        src_tab = np.vstack([src_tab, pad])
        dst_tab = np.vstack([dst_tab, pad])

    per_core = []
    counts = np.zeros((NCORES, 4), np.int64)
    for c in range(NCORES):
        sc = src[c * epc:(c + 1) * epc]
        dc = dst[c * epc:(c + 1) * epc]
        b = (sc >= nhalf) * 2 + (dc >= nhalf)
        ords = [np.flatnonzero(b == k) for k in range(4)]
        counts[c] = [len(o) for o in ords]
        per_core.append((sc, dc, ords))

    nb = [int(-(-counts[:, k].max() // B)) for k in range(4)]
    totb = sum(nb)

    idx_all = np.zeros((NCORES, 128, totb * NG * 128), np.int16)
    order_all = np.full((NCORES, totb * B), -1, np.int64)

    for c in range(NCORES):
        sc, dc, ords = per_core[c]
        t = 0
        pos = 0
        for k in range(4):
            ids = ords[k]
            cap = nb[k] * B
            se = np.zeros(cap, np.int64)
            de = np.zeros(cap, np.int64)
            se[:len(ids)] = sc[ids] - (nhalf if k >= 2 else 0)
            de[:len(ids)] = dc[ids] - (nhalf if k % 2 == 1 else 0)
            order_all[c, pos:pos + len(ids)] = ids
            pos += cap
            for bi in range(nb[k]):
                ib = t * NG * 128
                for h in range(NG):
                    sl = slice(bi * B + h * BG, bi * B + (h + 1) * BG)
                    idx_all[c, :, ib + h * 64:ib + (h + 1) * 64] = (
                        _wrap_idx_chunk(se[sl]))
                    idx_all[c, :, ib + NG * 64 + h * 64:
                            ib + NG * 64 + (h + 1) * 64] = (
                        _wrap_idx_chunk(de[sl]))
                t += 1

    W_mlp = np.asarray(inputs["W_mlp"], np.float32)
    b_mlp = np.asarray(inputs["b_mlp"], np.float32).ravel()
    wL = np.asarray(inputs["wL"], np.float32).ravel()

    wbd = np.zeros((128, 128), np.float16)
    wbd[:64, :64] = W_mlp.astype(np.float16)
    wbd[64:, 64:] = W_mlp.astype(np.float16)
    bm2 = np.concatenate([b_mlp, b_mlp]).astype(np.float32).reshape(128, 1)
    wlp2 = np.concatenate([wL, wL]).astype(np.float16)  # [128]
    wlpw = np.tile(wlp2[None, :], (128, NBLK))  # [128, NBLK*128]

    weights = dict(
        wbd=wbd, bm2=bm2, wlpw=wlpw,
        kb=np.full((128, 1), kb, np.float32),
    )
    tabs = dict(
        src_lo=np.ascontiguousarray(src_tab[:nhalf]),
        src_hi=np.ascontiguousarray(src_tab[nhalf:]),
        dst_lo=np.ascontiguousarray(dst_tab[:nhalf]),
        dst_hi=np.ascontiguousarray(dst_tab[nhalf:]),
    )
    return tuple(nb), nhalf, epc, E, tabs, weights, idx_all, order_all


def run(inputs, queues=2, **spmd_kwargs):
    from concourse.bass_utils import run_bass_kernel_spmd

    nb, nhalf, epc, E, tabs, weights, idx_all, order_all = _prep(inputs)

    key = (nb, nhalf, queues)
    if key not in _prog_cache:
        _prog_cache[key] = _build_program(list(nb), nhalf, queues=queues)
    nc = _prog_cache[key]

    in_maps = []
    for c in range(NCORES):
        m = dict(tabs)
        m.update(weights)
        m["idx"] = idx_all[c]
        in_maps.append(m)

    res = run_bass_kernel_spmd(nc, in_maps, list(range(NCORES)), **spmd_kwargs)

    out = np.empty((E, 1), np.float32)
    for c in range(NCORES):
        oc = np.asarray(res.results[c]["out"], np.float32)  # [totb*128, J]
        # edge e of batch t sits at oc[t*128 + e%128, e//128]
        totb = oc.shape[0] // 128
        oc = oc.reshape(totb, 128, J).transpose(0, 2, 1).reshape(-1)
        order = order_all[c]
        valid = order >= 0
        out[c * epc + order[valid], 0] = oc[valid]
    return out, res


def kernel(**inputs) -> np.ndarray:
    out, _ = run(inputs)
    return out
